# revision 13
# baseline (speedup 1.0000x reference)
"""3-layer GAT (GATConv) network on 8 Trainium2 NeuronCores.

Strategy (edge-parallel, dst-sharded):
  - Nodes are sharded evenly: core c owns nodes [c*1250, (c+1)*1250), padded
    to NP=1280 (10 tiles of 128).  Edges are sorted by dst and assigned to the
    core owning their dst node, grouped per 128-node dst tile, padded to a
    per-tile chunk count CH[t] shared across cores (same NEFF everywhere).
  - Per layer: each core computes the dense transforms for its own node shard
    (fused matmul [W | Wl | W@As | W@Ad]), packs a per-node "table row"
    [Hfeat bf16 | ALs fp32 | ALd fp32 | pad] and AllGathers the table.  The
    edge phase gathers table rows by src (GPSIMD dma_gather, one op per 6
    chunks), builds per-edge softmax numerators
    ex = exp(leaky_relu(ALs[src]+ALd[dst]+ALe)), scales the gathered features
    by ex, and aggregates per dst tile with a one-hot mask matmul (which also
    produces the softmax denominators from ex itself).  The output is
    normalized, skip+bias added, ELU applied, and kept in SBUF as the next
    layer's input (lhsT tiles come from PE transposes).  All pools live at
    top level so the scheduler can overlap the next dense phase with the
    current edge phase.  Inter-core traffic is only the 3 table AllGathers.
"""
import functools

import numpy as np
import ml_dtypes

import concourse.bass as bass
import concourse.bacc as bacc
import concourse.tile as tile
from concourse import mybir
from concourse.bass_utils import run_bass_kernel_spmd
from concourse.library_config import mlp
from concourse.masks import make_identity

P = 128
NCORES = 8
N = 10000
E = 160000
NSH = N // NCORES          # 1250 real nodes per core
NT = 10                    # dst tiles per core
NP = NT * P                # padded nodes per core (1280)
NTBL = NCORES * NP         # allgathered table rows
F_IN = 512
D12 = 1024                 # hidden width layers 1-2
H12, C12 = 4, 256
D3, H3, C3 = 96, 6, 16
NCLS = 16
RW12 = 1152                # table row lanes (bf16), 2304 B (256B multiple)
RW3 = 128                  # layer 3 row: 96 feat + 12 ALs + 12 ALd + 8 pad
GMAX = 6                   # chunks per dma_gather (SWDGE ring: <=768 rows/op)

f32 = mybir.dt.float32
bf16 = mybir.dt.bfloat16
i32 = mybir.dt.int32
i16 = mybir.dt.int16
u8 = mybir.dt.uint8
BF = ml_dtypes.bfloat16


def _build_nc(CH, use_collectives=True):
    TOTCH = int(sum(CH))
    EP = TOTCH * P
    nc = bacc.Bacc("TRN2", target_bir_lowering=False, debug=False,
                   num_devices=NCORES)
    # --- external I/O (per core) ---
    xt = nc.dram_tensor("xt", [F_IN, NP], bf16, kind="ExternalInput")
    src16 = nc.dram_tensor("src16", [P, EP // 16], i16, kind="ExternalInput")
    dst16 = nc.dram_tensor("dst16", [P, EP // 16], i16, kind="ExternalInput")
    dstl = nc.dram_tensor("dstl", [P, TOTCH], bf16, kind="ExternalInput")
    eat = nc.dram_tensor("eat", [6, EP], bf16, kind="ExternalInput")
    w1 = nc.dram_tensor("w1", [F_IN, 2056], bf16, kind="ExternalInput")
    w2 = nc.dram_tensor("w2", [D12, 2056], bf16, kind="ExternalInput")
    w3 = nc.dram_tensor("w3", [D12, D3 + NCLS + 2 * H3], bf16,
                        kind="ExternalInput")
    ve1 = nc.dram_tensor("ve1", [6, H12], bf16, kind="ExternalInput")
    ve2 = nc.dram_tensor("ve2", [6, H12], bf16, kind="ExternalInput")
    ve3 = nc.dram_tensor("ve3", [6, H3], bf16, kind="ExternalInput")
    brep1 = nc.dram_tensor("brep1", [P, D12], f32, kind="ExternalInput")
    brep2 = nc.dram_tensor("brep2", [P, D12], f32, kind="ExternalInput")
    brep3 = nc.dram_tensor("brep3", [P, NCLS], f32, kind="ExternalInput")
    outp = nc.dram_tensor("out", [NP, NCLS], f32, kind="ExternalOutput")
    # --- internal DRAM ---
    agin12 = nc.dram_tensor("agin12", [NP, RW12], bf16)
    agin3 = nc.dram_tensor("agin3", [NP, RW3], bf16)
    tbl12 = nc.dram_tensor("tbl12", [NTBL, RW12], bf16, addr_space="Shared")
    tbl3 = nc.dram_tensor("tbl3", [NTBL, RW3], bf16, addr_space="Shared")
    rg = [list(range(NCORES))]

    import contextlib
    with tile.TileContext(nc) as tc, contextlib.ExitStack() as cx:
        pool = lambda nm, bufs, **kw: cx.enter_context(
            tc.tile_pool(name=nm, bufs=bufs, **kw))
        cst = pool("cst", 1)
        wp = pool("wp", 8)
        lhs1p = pool("lhs1", 4)
        lhsp = pool("lhsp", 16)
        skp = pool("skp", 1)
        hp = pool("hp", 12)
        tabp = pool("tabp", 3)
        brp = pool("brp", 1)
        gp = pool("gp", 2)
        gpp = pool("gpp", 4)
        eatp = pool("eatp", 2)
        smp = pool("smp", 4)
        mkp = pool("mkp", 4)
        outp_pool = pool("outp", 2)
        # PSUM: m2 + al1 + agg2 + den1 + ale1 + tp1 = 8 banks
        psm = pool("psm", 2, space="PSUM")
        psal = pool("psal", 1, space="PSUM")
        psagg = pool("psagg", 2, space="PSUM")
        psden = pool("psden", 1, space="PSUM")
        psale = pool("psale", 1, space="PSUM")
        pstp = pool("pstp", 1, space="PSUM")

        nc.gpsimd.load_library(mlp)
        iota_row = cst.tile([P, P], bf16)
        nc.gpsimd.iota(iota_row[:], pattern=[[1, P]], base=0,
                       channel_multiplier=0,
                       allow_small_or_imprecise_dtypes=True)
        ident = cst.tile([P, P], bf16)
        make_identity(nc, ident[:])
        neg1 = cst.tile([P, 1], f32)
        nc.gpsimd.memset(neg1[:], -1.0)
        src16_sb = cst.tile([P, EP // 16], i16)
        nc.sync.dma_start(src16_sb[:], src16[:])
        dst16_sb = cst.tile([P, EP // 16], i16)
        nc.sync.dma_start(dst16_sb[:], dst16[:])
        dstl_sb = cst.tile([P, TOTCH], bf16)
        nc.sync.dma_start(dstl_sb[:], dstl[:])
        ve_sb = {}
        for nm, vd, hh in (("v1", ve1, H12), ("v2", ve2, H12), ("v3", ve3, H3)):
            v = cst.tile([6, hh], bf16, name=nm)
            nc.sync.dma_start(v[:], vd[:])
            ve_sb[nm] = v

        def allgather(agin, tbl):
            if use_collectives:
                nc.gpsimd.collective_compute(
                    "AllGather", mybir.AluOpType.bypass, replica_groups=rg,
                    ins=[agin[:]], outs=[tbl[:]])
            else:  # timing-sim stand-in
                for c in range(NCORES):
                    nc.sync.dma_start(tbl[c * NP:(c + 1) * NP, :], agin[:])

        # ------------------------------------------------------ dense phase
        def dense(K, wdram, brep_dram, lhs_get, skip_sb, tbl_agin,
                  Dmain, Dskip, HH, rw, als_lane):
            ms = Dmain + Dskip
            nj = (ms + 511) // 512
            Wcols = ms + 2 * HH
            brep = brp.tile([P, Dskip], f32, tag="br")
            nc.sync.dma_start(brep[:], brep_dram[:])
            wsb = []
            for k in range(K):
                w = wp.tile([P, Wcols], bf16, tag="w")
                nc.sync.dma_start(w[:], wdram[k * P:(k + 1) * P, :])
                wsb.append(w)
            for t in range(NT):
                lhsT = [lhs_get(k, t) for k in range(K)]
                pal = psal.tile([P, 2 * HH], f32, space="PSUM", tag="al")
                tabt = tabp.tile([P, rw], bf16, tag="t")
                for j0 in range(0, nj, 2):
                    js = list(range(j0, min(j0 + 2, nj)))
                    ps = {}
                    for j in js:
                        width = min(512, ms - j * 512)
                        ps[j] = psm.tile([P, width], f32, space="PSUM",
                                         tag="m", name=f"dps{j}")
                    for k in range(K):
                        first, last = (k == 0), (k == K - 1)
                        for j in js:
                            width = min(512, ms - j * 512)
                            nc.tensor.matmul(
                                ps[j][:], lhsT=lhsT[k],
                                rhs=wsb[k][:, j * 512:j * 512 + width],
                                start=first, stop=last)
                        if j0 == 0:
                            nc.tensor.matmul(pal[:], lhsT=lhsT[k],
                                             rhs=wsb[k][:, ms:ms + 2 * HH],
                                             start=first, stop=last)
                    # drain this half into table / skip storage
                    for j in js:
                        width = min(512, ms - j * 512)
                        lo, hi = j * 512, j * 512 + width
                        if lo < Dmain:  # feature part -> table (bf16)
                            w2_ = min(hi, Dmain) - lo
                            nc.scalar.activation(
                                tabt[:, lo:lo + w2_], ps[j][:, 0:w2_],
                                mybir.ActivationFunctionType.Copy)
                            lo2 = lo + w2_
                        else:
                            lo2 = lo
                        if hi > Dmain:  # skip part -> skip_sb (+bias, bf16)
                            o = lo2 - Dmain
                            w3_ = hi - lo2
                            nc.vector.tensor_tensor(
                                out=skip_sb[:, t * Dskip + o:
                                            t * Dskip + o + w3_],
                                in0=ps[j][:, lo2 - lo:lo2 - lo + w3_],
                                in1=brep[:, o:o + w3_],
                                op=mybir.AluOpType.add)
                nc.vector.tensor_copy(
                    tabt[:, als_lane:als_lane + 4 * HH].bitcast(f32), pal[:])
                nc.sync.dma_start(tbl_agin[t * P:(t + 1) * P, :], tabt[:])

        # ------------------------------------------------------- edge phase
        def edge(lnum, tbl, ve, skip_sb, Dmain, HH, CC, rw,
                 als_f0, ald_f0, a_off, out_writer):
            nja = (Dmain + 511) // 512
            gofs = 0
            for t in range(NT):
                cht = CH[t]
                eat_t = eatp.tile([6, cht * P], bf16, tag="eat")
                nc.sync.dma_start(eat_t[:], eat[:, gofs * P:(gofs + cht) * P])
                agg = []
                for j in range(nja):
                    width = min(512, Dmain - j * 512)
                    agg.append(psagg.tile([P, width], f32, space="PSUM",
                                          tag="agg", name=f"agg{j}"))
                den = psden.tile([P, HH], f32, space="PSUM", tag="den")
                for j0 in range(0, cht, GMAX):
                    gk = min(GMAX, cht - j0)
                    jg = gofs + j0
                    G = gp.tile([P, gk * rw], bf16, tag="G")
                    nc.gpsimd.dma_gather(
                        out_ap=G[:].rearrange("p (k r) -> p k r", k=gk),
                        in_ap=bass.AP(tbl, 0, [[rw, NTBL], [1, rw]]),
                        idxs_ap=src16_sb[:, jg * 8:(jg + gk) * 8],
                        num_idxs=gk * P, num_idxs_reg=gk * P,
                        elem_size=rw, elem_step=rw)
                    A = gp.tile([P, gk * 128], bf16, tag="A")
                    nc.gpsimd.dma_gather(
                        out_ap=A[:].rearrange("p (k r) -> p k r", k=gk),
                        in_ap=bass.AP(tbl, a_off, [[rw, NTBL], [1, 128]]),
                        idxs_ap=dst16_sb[:, jg * 8:(jg + gk) * 8],
                        num_idxs=gk * P, num_idxs_reg=gk * P,
                        elem_size=128, elem_step=rw)
                    pae = psale.tile([P, gk * HH], f32, space="PSUM",
                                     tag="ale")
                    for j in range(gk):
                        nc.tensor.matmul(
                            pae[:, j * HH:(j + 1) * HH],
                            lhsT=eat_t[:, (j0 + j) * P:(j0 + j + 1) * P],
                            rhs=ve[:], start=True, stop=True)
                    Gf = G[:].bitcast(f32).rearrange("p (k r) -> p k r", k=gk)
                    Af = A[:].bitcast(f32).rearrange("p (k r) -> p k r", k=gk)
                    lg = smp.tile([P, gk * HH], f32, tag="lg")
                    nc.vector.tensor_tensor(
                        out=lg[:].rearrange("p (k r) -> p k r", k=gk),
                        in0=Gf[:, :, als_f0:als_f0 + HH],
                        in1=Af[:, :, ald_f0:ald_f0 + HH],
                        op=mybir.AluOpType.add)
                    nc.vector.tensor_tensor(out=lg[:], in0=lg[:], in1=pae[:],
                                            op=mybir.AluOpType.add)
                    lk = smp.tile([P, gk * HH], f32, tag="lk")
                    nc.vector.tensor_scalar_mul(lk[:], lg[:], 0.2)
                    nc.vector.tensor_tensor(out=lk[:], in0=lg[:], in1=lk[:],
                                            op=mybir.AluOpType.max)
                    ex = smp.tile([P, gk * HH], f32, tag="ex")
                    nc.scalar.activation(ex[:], lk[:],
                                         mybir.ActivationFunctionType.Exp)
                    exb = smp.tile([P, gk * HH], bf16, tag="exb")
                    nc.scalar.activation(exb[:], ex[:],
                                         mybir.ActivationFunctionType.Copy)
                    for j in range(gk):
                        ch = j0 + j
                        first, last = (ch == 0), (ch == cht - 1)
                        mask = mkp.tile([P, P], bf16, tag="mk")
                        nc.vector.tensor_tensor(
                            out=mask[:],
                            in0=dstl_sb[:, jg + j:jg + j + 1
                                        ].to_broadcast([P, P]),
                            in1=iota_row[:], op=mybir.AluOpType.is_equal)
                        Gp = gpp.tile([P, Dmain], bf16, tag="Gp")
                        if CC >= 128:
                            for h in range(HH):
                                nc.vector.tensor_scalar_mul(
                                    Gp[:, h * CC:(h + 1) * CC],
                                    G[:, j * rw + h * CC:
                                      j * rw + (h + 1) * CC],
                                    ex[:, j * HH + h:j * HH + h + 1])
                        else:  # layer 3: one mult with head-broadcast ex
                            exs = exb[:, j * HH:(j + 1) * HH]
                            exbc = bass.AP(exs.tensor, exs.offset,
                                           [exs.ap[0], [1, HH], [0, CC]])
                            nc.vector.tensor_tensor(
                                out=Gp[:, 0:Dmain].rearrange(
                                    "p (h c) -> p h c", h=HH),
                                in0=G[:, j * rw:j * rw + Dmain].rearrange(
                                    "p (h c) -> p h c", h=HH),
                                in1=exbc, op=mybir.AluOpType.mult)
                        for jj in range(nja):
                            width = min(512, Dmain - jj * 512)
                            nc.tensor.matmul(
                                agg[jj][:], lhsT=mask[:],
                                rhs=Gp[:, jj * 512:jj * 512 + width],
                                start=first, stop=last)
                        nc.tensor.matmul(den[:], lhsT=mask[:],
                                         rhs=exb[:, j * HH:(j + 1) * HH],
                                         start=first, stop=last)
                gofs += cht
                deni = smp.tile([P, HH], f32, tag="deni")
                nc.vector.tensor_scalar_add(deni[:], den[:], 1e-30)
                rec = smp.tile([P, HH], f32, tag="rec")
                nc.vector.reciprocal(rec[:], deni[:])
                out_writer(t, agg, rec, skip_sb)

        # ------------------------------------------------- writers / lhsT
        def make_h_writer(h_tiles):
            def writer(t, agg, rec, skip_sb):
                pre = outp_pool.tile([P, D12], f32, tag="pre")
                for h in range(H12):
                    j, off = (h * C12) // 512, (h * C12) % 512
                    nc.vector.tensor_scalar_mul(
                        pre[:, h * C12:(h + 1) * C12],
                        agg[j][:, off:off + C12], rec[:, h:h + 1])
                nc.vector.tensor_tensor(
                    out=pre[:], in0=pre[:],
                    in1=skip_sb[:, t * D12:(t + 1) * D12],
                    op=mybir.AluOpType.add)
                expd = outp_pool.tile([P, D12], f32, tag="expd")
                nc.scalar.activation(expd[:], pre[:],
                                     mybir.ActivationFunctionType.Exp)
                nc.vector.tensor_scalar_add(expd[:], expd[:], -1.0)
                mgt = outp_pool.tile([P, D12], u8, tag="mgt")
                nc.vector.tensor_scalar(mgt[:], pre[:], 0.0, None,
                                        op0=mybir.AluOpType.is_gt)
                ht = hp.tile([P, D12], bf16, tag="ht", name=f"h{t}")
                nc.vector.select(ht[:], mgt[:], pre[:], expd[:])
                h_tiles.append(ht)
            return writer

        def out3_writer(t, agg, rec, skip_sb):
            rec6 = smp.tile([P, H3], f32, tag="rec6")
            nc.vector.tensor_scalar_mul(rec6[:], rec[:], 1.0 / H3)
            acc = outp_pool.tile([P, NCLS], f32, tag="acc3")
            nc.vector.tensor_scalar_mul(acc[:], agg[0][:, 0:NCLS],
                                        rec6[:, 0:1])
            for h in range(1, H3):
                tmp = outp_pool.tile([P, NCLS], f32, tag="tmp3")
                nc.vector.tensor_scalar_mul(
                    tmp[:], agg[0][:, h * NCLS:(h + 1) * NCLS],
                    rec6[:, h:h + 1])
                nc.vector.tensor_tensor(out=acc[:], in0=acc[:], in1=tmp[:],
                                        op=mybir.AluOpType.add)
            nc.vector.tensor_tensor(
                out=acc[:], in0=acc[:],
                in1=skip_sb[:, t * NCLS:(t + 1) * NCLS],
                op=mybir.AluOpType.add)
            nc.sync.dma_start(outp[t * P:(t + 1) * P, :], acc[:])

        def lhs_from_h(h_tiles):
            cache = {}
            def get(k, t):
                if (k, t) not in cache:
                    tp = pstp.tile([P, P], bf16, space="PSUM", tag="tp",
                                   name=f"tp{k}_{t}")
                    nc.tensor.transpose(tp[:],
                                        h_tiles[t][:, k * P:(k + 1) * P],
                                        ident[:])
                    lt = lhsp.tile([P, P], bf16, tag="lh", name=f"lh{k}_{t}")
                    nc.scalar.activation(lt[:], tp[:],
                                         mybir.ActivationFunctionType.Copy)
                    cache[(k, t)] = lt
                return cache[(k, t)][:]
            return get

        # ---------------- layer 1 ----------------
        lhs1 = []
        for k in range(F_IN // P):
            lt = lhs1p.tile([P, NP], bf16, tag="x")
            nc.sync.dma_start(lt[:], xt[k * P:(k + 1) * P, :])
            lhs1.append(lt)
        skip1 = skp.tile([P, NT * D12], bf16, tag="sk12")
        dense(F_IN // P, w1, brep1, lambda k, t: lhs1[k][:, t * P:(t + 1) * P],
              skip1, agin12, D12, D12, H12, RW12, 1024)
        allgather(agin12, tbl12)
        h1_tiles = []
        edge(1, tbl12, ve_sb["v1"][:], skip1, D12, H12, C12, RW12,
             512, 4, 1024, make_h_writer(h1_tiles))

        # ---------------- layer 2 ----------------
        skip2 = skp.tile([P, NT * D12], bf16, tag="sk12b")
        dense(D12 // P, w2, brep2, lhs_from_h(h1_tiles),
              skip2, agin12, D12, D12, H12, RW12, 1024)
        allgather(agin12, tbl12)
        h2_tiles = []
        edge(2, tbl12, ve_sb["v2"][:], skip2, D12, H12, C12, RW12,
             512, 4, 1024, make_h_writer(h2_tiles))

        # ---------------- layer 3 ----------------
        skip3 = skp.tile([P, NT * NCLS], bf16, tag="sk3")
        dense(D12 // P, w3, brep3, lhs_from_h(h2_tiles),
              skip3, agin3, D3, NCLS, H3, RW3, 96)
        allgather(agin3, tbl3)
        edge(3, tbl3, ve_sb["v3"][:], skip3, D3, H3, C3, RW3,
             48, 54, 0, out3_writer)

    nc.compile()
    return nc


@functools.lru_cache(maxsize=2)
def _built(CH_tuple):
    return _build_nc(list(CH_tuple))


# ------------------------------------------------------------------ host prep

def _fold_as(W, a):
    H, C = a.shape
    return np.einsum('dhc,hc->dh', W.reshape(W.shape[0], H, C), a)


def _prep(inputs):
    x = np.asarray(inputs['x'], np.float32)
    ei = np.asarray(inputs['edge_index'], np.int64)
    ea = np.asarray(inputs['edge_attr'], np.float32)
    src, dst = ei[0], ei[1]

    order = np.argsort(dst, kind='stable')
    src, dst = src[order], dst[order]
    ea_s = ea[order]

    core_of = dst // NSH
    loc = dst - core_of * NSH
    tile_of = loc // P

    cnt = np.zeros((NCORES, NT), np.int64)
    for c in range(NCORES):
        m = core_of == c
        cnt[c] = np.bincount(tile_of[m], minlength=NT)
    CH = tuple(int(v) for v in np.ceil(cnt.max(axis=0) / P).astype(np.int64))
    TOTCH = int(sum(CH))
    EP = TOTCH * P

    pos = (src // NSH) * NP + (src % NSH)
    posd = (dst // NSH) * NP + (dst % NSH)

    def wrap16(a):  # flat [EP] -> [128, EP//16] int16 (16-wrap, 8x replicate)
        return np.tile(np.ascontiguousarray(a.reshape(-1, 16).T), (8, 1)
                       ).astype(np.int16)

    per_core = []
    for c in range(NCORES):
        sg = np.zeros(EP, np.int64)
        dg = np.zeros(EP, np.int64)
        dl = np.full(EP, -1, np.int64)
        et = np.zeros((EP, 6), np.float32)
        off = 0
        m = core_of == c
        for t in range(NT):
            mt = m & (tile_of == t)
            k = int(mt.sum())
            sl = slice(off, off + k)
            sg[sl] = pos[mt]
            dg[sl] = posd[mt]
            dl[sl] = loc[mt] - t * P
            et[sl] = ea_s[mt]
            off += CH[t] * P
        xtc = np.zeros((F_IN, NP), np.float32)
        xtc[:, :NSH] = x[c * NSH:(c + 1) * NSH].T
        per_core.append(dict(
            xt=xtc.astype(BF),
            src16=wrap16(sg), dst16=wrap16(dg),
            dstl=np.ascontiguousarray(dl.reshape(TOTCH, P).T).astype(BF),
            eat=np.ascontiguousarray(et.T).astype(BF),
        ))

    def wext(W, Wl, a_s, a_d):
        return np.concatenate(
            [W, Wl, _fold_as(W, a_s), _fold_as(W, a_d)], axis=1).astype(BF)

    i = {k: np.asarray(v, np.float32) for k, v in inputs.items()
         if k not in ('edge_index',)}
    shared = dict(
        w1=wext(i['W1'], i['Wl1'], i['as1'], i['ad1']),
        w2=wext(i['W2'], i['Wl2'], i['as2'], i['ad2']),
        w3=wext(i['W3'], i['Wl3'], i['as3'], i['ad3']),
        ve1=_fold_as(i['We1'], i['ae1']).astype(BF),
        ve2=_fold_as(i['We2'], i['ae2']).astype(BF),
        ve3=_fold_as(i['We3'], i['ae3']).astype(BF),
        brep1=np.repeat((i['b1'] + i['bl1'])[None, :], P, 0).astype(np.float32),
        brep2=np.repeat((i['b2'] + i['bl2'])[None, :], P, 0).astype(np.float32),
        brep3=np.repeat((i['b3'] + i['bl3'])[None, :], P, 0).astype(np.float32),
    )
    in_maps = [dict(per_core[c], **shared) for c in range(NCORES)]
    return CH, in_maps


def kernel(**inputs):
    CH, in_maps = _prep(inputs)
    nc = _built(CH)
    res = run_bass_kernel_spmd(nc, in_maps, core_ids=list(range(NCORES)))
    out = np.empty((N, NCLS), np.float32)
    for c in range(NCORES):
        out[c * NSH:(c + 1) * NSH] = res.results[c]["out"][:NSH]
    return out


if __name__ == "__main__":
    inp = np.load('/tmp/inputs.npy', allow_pickle=True).item()
    got = kernel(**inp)
    ref = np.load('/tmp/ref_out.npy')
    d = np.abs(got - ref)
    s = np.abs(ref).max()
    print("absmax err:", d.max(), "scale:", s, "rel:", d.max() / s)
